# revision 73
# baseline (speedup 1.0000x reference)
import sys

sys.path.insert(0, "/opt/trn_rl_repo")
import time
from contextlib import ExitStack

import numpy as np
import ml_dtypes

import concourse.bass as bass
import concourse.mybir as mybir
import concourse.tile as tile
from concourse import bacc
from concourse.bass_utils import run_bass_kernel_spmd
from concourse.masks import make_identity

H = 1024
I = 2048
E = 8
TOP_K = 2
CAP_FACTOR = 1.25
RMS_EPS = 1e-6
BIT_EPS = 1e-8

N_TOK = 8192            # 4 * 2048
CAP = 1280              # int(N_TOK / E * CAP_FACTOR)
NS_TILES = 8            # shared-phase token tiles per core (1024 tokens)
NE_TILES = CAP // 128   # expert-phase token tiles per core (1280 tokens)
NT = NS_TILES + NE_TILES
P = 128
MAGIC = 12582912.0      # 1.5 * 2**23: fp32 add forces round-half-even to int

F32 = mybir.dt.float32
BF16 = mybir.dt.bfloat16
AX = mybir.AxisListType
ALU = mybir.AluOpType
ACT = mybir.ActivationFunctionType

LAST_EXEC_NS = None
LAST_WALL_NS = None
LAST_IN_MAPS = None


# --------------------------------------------------------------------------
# host-side helpers (router + weight ternarization + marshalling)
# --------------------------------------------------------------------------

def _ternarize(w):
    """BitNet absmean weight quant: returns ternary {-1,0,1} and the
    multiplier 1/scale so that wq = tern * mult."""
    w = np.asarray(w, np.float32)
    scale = np.float32(1.0) / np.clip(np.mean(np.abs(w)), 1e-5, None).astype(np.float32)
    tern = np.clip(np.round(w * scale), -1.0, 1.0).astype(np.float32)
    return tern, np.float32(1.0 / scale)


def _router(x_flat, gate_norm_w, gate_w):
    """Replicates the reference router in fp32 numpy."""
    xn = x_flat / np.sqrt(np.mean(x_flat * x_flat, axis=-1, keepdims=True) + RMS_EPS)
    xn = (xn * gate_norm_w).astype(np.float32)
    logits = xn @ gate_w.T
    m = logits.max(axis=-1, keepdims=True)
    ex = np.exp(logits - m)
    probs = (ex / ex.sum(axis=-1, keepdims=True)).astype(np.float32)
    order = np.argsort(-probs, axis=1, kind="stable")
    top_idx = order[:, :TOP_K]
    top_w = np.take_along_axis(probs, top_idx, axis=1)

    n = x_flat.shape[0]
    expert_mask = np.zeros((n, E), np.float32)
    expert_mask[np.arange(n)[:, None], top_idx] = top_w
    w_keep = np.zeros((n, E), np.float32)
    for e in range(E):
        sel = expert_mask[:, e] > 0
        keep = sel & (np.cumsum(sel.astype(np.int64)) <= CAP)
        w_keep[:, e] = np.where(keep, expert_mask[:, e], 0.0)
    return w_keep


# --------------------------------------------------------------------------
# device kernel (one program, SPMD across 8 cores)
# --------------------------------------------------------------------------

def _build_nc():
    nc = bacc.Bacc("TRN2", target_bir_lowering=False, debug=False)

    x_d = nc.dram_tensor("x", [NT, P, H], F32, kind="ExternalInput")
    ga_d = nc.dram_tensor("ga", [NT, P, 1], F32, kind="ExternalInput")
    ep_d = nc.dram_tensor("ep", [NT, P, 1], F32, kind="ExternalInput")
    wg_d = nc.dram_tensor("wg", [2, 8, P, 2 * I], BF16, kind="ExternalInput")
    wd_d = nc.dram_tensor("wd", [2, 16, P, H], BF16, kind="ExternalInput")
    o_d = nc.dram_tensor("o", [NT, P, H], F32, kind="ExternalOutput")

    with tile.TileContext(nc) as tc, ExitStack() as ctx:
        constp = ctx.enter_context(tc.tile_pool(name="constp", bufs=1))
        wgp = ctx.enter_context(tc.tile_pool(name="wgp", bufs=18))
        wdp = ctx.enter_context(tc.tile_pool(name="wdp", bufs=17))
        xp = ctx.enter_context(tc.tile_pool(name="xp", bufs=3))
        scrp = ctx.enter_context(tc.tile_pool(name="scrp", bufs=3))
        xqp = ctx.enter_context(tc.tile_pool(name="xqp", bufs=2))
        xqtp = ctx.enter_context(tc.tile_pool(name="xqtp", bufs=2))
        pp = ctx.enter_context(tc.tile_pool(name="pp", bufs=2))
        hqp = ctx.enter_context(tc.tile_pool(name="hqp", bufs=3))
        hqtp = ctx.enter_context(tc.tile_pool(name="hqtp", bufs=2))
        silup = ctx.enter_context(tc.tile_pool(name="silup", bufs=2))
        outp = ctx.enter_context(tc.tile_pool(name="outp", bufs=2))
        rowp = ctx.enter_context(tc.tile_pool(name="rowp", bufs=3))
        mm1p = ctx.enter_context(tc.tile_pool(name="mm1p", bufs=4, space="PSUM"))
        mm2p = ctx.enter_context(tc.tile_pool(name="mm2p", bufs=2, space="PSUM"))
        tpp = ctx.enter_context(tc.tile_pool(name="tpp", bufs=2, space="PSUM"))

        # weight tiles per phase (phase 0 = shared expert, 1 = routed expert)
        wg_sb_all = [[], []]
        wd_sb_all = [[], []]

        def alloc_wg(phase):
            for k in range(8):
                wt_g = wgp.tile([P, I], BF16, tag="wg")
                wt_y = wgp.tile([P, I], BF16, tag="wg")
                wg_sb_all[phase].append((wt_g, wt_y))

        def load_wg(phase):
            if not wg_sb_all[phase]:
                alloc_wg(phase)
            engs = [nc.sync, nc.scalar, nc.gpsimd]
            for k in range(8):
                wt_g, wt_y = wg_sb_all[phase][k]
                engs[(2 * k) % 3].dma_start(out=wt_g, in_=wg_d[phase, k][:, 0:I])
                engs[(2 * k + 1) % 3].dma_start(out=wt_y, in_=wg_d[phase, k][:, I:2 * I])

        def load_wd(phase):
            for k in range(16):
                wt = wdp.tile([P, H], BF16, tag="wd")
                nc.gpsimd.dma_start(out=wt, in_=wd_d[phase, k])
                wd_sb_all[phase].append(wt)

        alloc_wg(0)

        ident = constp.tile([P, P], BF16)
        make_identity(nc, ident)
        warm = constp.tile([P, 1], F32)
        nc.vector.memset(warm, 1.0)
        nc.scalar.activation(out=warm, in_=warm, func=ACT.Square)

        I32 = mybir.dt.int32

        def fisr(src, tag):
            """1/sqrt(src) for positive fp32 [P,1] on GPSIMD (keeps the DVE
            stream clear). Magic seed + 2 Newton steps (rel err ~3e-7)."""
            eng = nc.vector
            si = src.bitcast(I32)
            t0 = rowp.tile([P, 1], I32, tag=tag + "_i")
            eng.tensor_scalar(out=t0, in0=si, scalar1=1, scalar2=None,
                              op0=ALU.arith_shift_right)
            y = rowp.tile([P, 1], I32, tag=tag + "_y")
            eng.tensor_scalar(out=y, in0=t0, scalar1=-1,
                              scalar2=0x5F3759DF, op0=ALU.mult, op1=ALU.add)
            yf = y.bitcast(F32)
            for it in range(2):
                a = rowp.tile([P, 1], F32, tag=tag + f"_a{it}")
                eng.tensor_tensor(out=a, in0=yf, in1=yf, op=ALU.mult)
                b = rowp.tile([P, 1], F32, tag=tag + f"_b{it}")
                eng.tensor_tensor(out=b, in0=a, in1=src, op=ALU.mult)
                c = rowp.tile([P, 1], F32, tag=tag + f"_c{it}")
                eng.tensor_scalar(out=c, in0=b, scalar1=-0.5, scalar2=1.5,
                                  op0=ALU.mult, op1=ALU.add)
                y2 = rowp.tile([P, 1], F32, tag=tag + f"_n{it}")
                eng.tensor_tensor(out=y2, in0=yf, in1=c, op=ALU.mult)
                yf = y2
            return yf

        def stage_a_pre(t):
            """x-load, x-quant, xq transpose (no weight reads)."""

            x_sb = xp.tile([P, H], F32, tag="x")
            nc.gpsimd.dma_start(out=x_sb, in_=x_d[t])
            ga_sb = rowp.tile([P, 1], F32, tag="ga")
            nc.gpsimd.dma_start(out=ga_sb, in_=ga_d[t])
            ep_sb = rowp.tile([P, 1], F32, tag="ep")
            nc.gpsimd.dma_start(out=ep_sb, in_=ep_d[t])

            # ---- x absmax-quant (per token, exact BitNet semantics) ----
            m_x = rowp.tile([P, 1], F32, tag="m_x")
            nc.vector.tensor_reduce(
                out=m_x, in_=x_sb, axis=AX.X, op=ALU.max,
                apply_absolute_value=True,
            )
            scr_x = scrp.tile([P, H], F32, tag="scr")
            ssq_x = rowp.tile([P, 1], F32, tag="ssq_x")
            nc.scalar.activation(out=scr_x, in_=x_sb, func=ACT.Square,
                                 accum_out=ssq_x)
            ms_x = rowp.tile([P, 1], F32, tag="ms_x")
            nc.vector.tensor_scalar(out=ms_x, in0=ssq_x, scalar1=1.0 / H,
                                    scalar2=BIT_EPS, op0=ALU.mult, op1=ALU.add)
            # quant scale: 127/max|x| (exact except for identically-tiny rows,
            # which quantize to zero either way); rsqrt path only feeds the
            # sigmoid/epilogue scales and so runs off the critical path.
            mc_x = rowp.tile([P, 1], F32, tag="mc_x")
            nc.vector.tensor_scalar(out=mc_x, in0=m_x, scalar1=1e-20,
                                    scalar2=None, op0=ALU.max)
            mi_x = rowp.tile([P, 1], F32, tag="mi_x")
            nc.vector.reciprocal(out=mi_x, in_=mc_x)
            q_x = rowp.tile([P, 1], F32, tag="q_x")
            nc.vector.tensor_scalar(out=q_x, in0=mi_x, scalar1=127.0,
                                    scalar2=None, op0=ALU.mult)
            u_x = fisr(ms_x[:, 0:1], "ux")
            v_x = rowp.tile([P, 1], F32, tag="v_x")
            nc.vector.tensor_tensor(out=v_x, in0=u_x, in1=m_x, op=ALU.mult)
            c_x = rowp.tile([P, 1], F32, tag="c_x")
            nc.vector.tensor_scalar(out=c_x, in0=v_x, scalar1=1e-5,
                                    scalar2=None, op0=ALU.max)
            s_comb = rowp.tile([P, 1], F32, tag="s_comb")
            nc.vector.tensor_scalar(out=s_comb, in0=c_x, scalar1=ga_sb[:, 0:1],
                                    scalar2=None, op0=ALU.mult)

            tq_x = scrp.tile([P, H], F32, tag="scr")
            nc.vector.tensor_scalar(out=tq_x, in0=x_sb, scalar1=q_x[:, 0:1],
                                    scalar2=MAGIC, op0=ALU.mult, op1=ALU.add)
            xq = xqp.tile([P, H], BF16, tag="xq")
            nc.scalar.activation(out=xq, in_=tq_x, func=ACT.Copy, bias=-MAGIC)

            # ---- transpose xq -> H-major for mm1 stationary ----
            xqT = xqtp.tile([P, 8, P], BF16, tag="xqT")
            for g2 in range(2):
                tp = tpp.tile([P, 512], BF16, tag="tp")
                for j in range(4):
                    jj = g2 * 4 + j
                    nc.tensor.transpose(
                        out=tp[:, j * P:(j + 1) * P],
                        in_=xq[:, jj * P:(jj + 1) * P],
                        identity=ident,
                    )
                nc.vector.tensor_copy(out=xqT[:, g2 * 4:(g2 + 1) * 4, :], in_=tp)

            return xqT, s_comb, ep_sb

        def stage_a(t, pre):
            """mm1, h math, hq quant. Returns state needed by stage_b."""
            phase = 0 if t < NS_TILES else 1
            wg_sb = wg_sb_all[phase]
            xqT, s_comb, ep_sb = pre

            # ---- mm1 + sigmoid path; p2 = g_int*sigmoid(g_true)*y_int ----
            p_sb = pp.tile([P, I], F32, tag="p")
            scr_p = scrp.tile([P, I], F32, tag="scr")
            m_parts, sq_parts = [], []
            for jh in range(4):
                g_ps = mm1p.tile([P, 512], F32, tag="mm1")
                y_ps = mm1p.tile([P, 512], F32, tag="mm1")
                for k in range(8):
                    nc.tensor.matmul(
                        g_ps, lhsT=xqT[:, k, :],
                        rhs=wg_sb[k][0][:, jh * 512:(jh + 1) * 512],
                        start=(k == 0), stop=(k == 7),
                    )
                for k in range(8):
                    nc.tensor.matmul(
                        y_ps, lhsT=xqT[:, k, :],
                        rhs=wg_sb[k][1][:, jh * 512:(jh + 1) * 512],
                        start=(k == 0), stop=(k == 7),
                    )
                sg = silup.tile([P, 512], F32, tag="sg")
                nc.scalar.activation(out=sg, in_=g_ps, func=ACT.Sigmoid,
                                     scale=s_comb[:, 0:1])
                nc.vector.tensor_tensor(out=sg, in0=sg, in1=g_ps, op=ALU.mult)
                sl = p_sb[:, jh * 512:(jh + 1) * 512]
                nc.vector.tensor_tensor(out=sl, in0=sg, in1=y_ps, op=ALU.mult)
                m_j = rowp.tile([P, 1], F32, tag="m_j")
                nc.vector.tensor_reduce(out=m_j, in_=sl, axis=AX.X, op=ALU.max,
                                        apply_absolute_value=True)
                m_parts.append(m_j)
                sq_j = rowp.tile([P, 1], F32, tag="sq_j")
                nc.scalar.activation(out=scr_p[:, jh * 512:(jh + 1) * 512],
                                     in_=sl, func=ACT.Square, accum_out=sq_j)
                sq_parts.append(sq_j)

            m_a = rowp.tile([P, 1], F32, tag="m_a")
            nc.vector.tensor_tensor(out=m_a, in0=m_parts[0], in1=m_parts[1],
                                    op=ALU.max)
            m_b = rowp.tile([P, 1], F32, tag="m_b")
            nc.vector.tensor_tensor(out=m_b, in0=m_parts[2], in1=m_parts[3],
                                    op=ALU.max)
            m_p = rowp.tile([P, 1], F32, tag="m_p")
            nc.vector.tensor_tensor(out=m_p, in0=m_a, in1=m_b, op=ALU.max)
            s_a = rowp.tile([P, 1], F32, tag="s_a")
            nc.vector.tensor_tensor(out=s_a, in0=sq_parts[0], in1=sq_parts[1],
                                    op=ALU.add)
            s_b2 = rowp.tile([P, 1], F32, tag="s_b2")
            nc.vector.tensor_tensor(out=s_b2, in0=sq_parts[2], in1=sq_parts[3],
                                    op=ALU.add)
            ssq_p = rowp.tile([P, 1], F32, tag="ssq_p")
            nc.vector.tensor_tensor(out=ssq_p, in0=s_a, in1=s_b2, op=ALU.add)

            # ---- h-quant scales (exact, including the 1e-5 clip) ----
            # h = p2 * s_comb^2; rrms_h = rsqrt(mean(h^2)+eps)
            sc2 = rowp.tile([P, 1], F32, tag="sc2")
            nc.vector.tensor_tensor(out=sc2, in0=s_comb, in1=s_comb, op=ALU.mult)
            sc4 = rowp.tile([P, 1], F32, tag="sc4")
            nc.vector.tensor_tensor(out=sc4, in0=sc2, in1=sc2, op=ALU.mult)
            t1 = rowp.tile([P, 1], F32, tag="t1")
            nc.vector.tensor_tensor(out=t1, in0=ssq_p, in1=sc4, op=ALU.mult)
            ms_h = rowp.tile([P, 1], F32, tag="ms_h")
            nc.vector.tensor_scalar(out=ms_h, in0=t1, scalar1=1.0 / I,
                                    scalar2=BIT_EPS, op0=ALU.mult, op1=ALU.add)
            mc_h = rowp.tile([P, 1], F32, tag="mc_h")
            nc.vector.tensor_scalar(out=mc_h, in0=m_p, scalar1=1e-20,
                                    scalar2=None, op0=ALU.max)
            mi_h = rowp.tile([P, 1], F32, tag="mi_h")
            nc.vector.reciprocal(out=mi_h, in_=mc_h)
            q_h = rowp.tile([P, 1], F32, tag="q_h")
            nc.vector.tensor_scalar(out=q_h, in0=mi_h, scalar1=127.0,
                                    scalar2=None, op0=ALU.mult)
            rr_h = fisr(ms_h[:, 0:1], "rrh")
            u_h = rowp.tile([P, 1], F32, tag="u_h")
            nc.vector.tensor_tensor(out=u_h, in0=sc2, in1=rr_h, op=ALU.mult)
            v_h = rowp.tile([P, 1], F32, tag="v_h")
            nc.vector.tensor_tensor(out=v_h, in0=u_h, in1=m_p, op=ALU.mult)
            c_h = rowp.tile([P, 1], F32, tag="c_h")
            nc.vector.tensor_scalar(out=c_h, in0=v_h, scalar1=1e-5,
                                    scalar2=None, op0=ALU.max)
            stot = rowp.tile([P, 1], F32, tag="stot")
            nc.vector.tensor_scalar(out=stot, in0=c_h, scalar1=ep_sb[:, 0:1],
                                    scalar2=None, op0=ALU.mult)

            tq_h = scrp.tile([P, I], F32, tag="scr")
            nc.vector.tensor_scalar(out=tq_h, in0=p_sb, scalar1=q_h[:, 0:1],
                                    scalar2=MAGIC, op0=ALU.mult, op1=ALU.add)
            hq = hqp.tile([P, I], BF16, tag="hq")
            nc.scalar.activation(out=hq, in_=tq_h, func=ACT.Copy, bias=-MAGIC)
            return hq, stot

        def stage_b(t, hq, stot):
            """hq transpose, mm2, epilogue scale, store."""
            phase = 0 if t < NS_TILES else 1
            wd_sb = wd_sb_all[phase]

            hqT = hqtp.tile([P, 16, P], BF16, tag="hqT")
            for g4 in range(4):
                tp = tpp.tile([P, 512], BF16, tag="tp")
                for j in range(4):
                    jj = g4 * 4 + j
                    nc.tensor.transpose(
                        out=tp[:, j * P:(j + 1) * P],
                        in_=hq[:, jj * P:(jj + 1) * P],
                        identity=ident,
                    )
                nc.vector.tensor_copy(out=hqT[:, g4 * 4:(g4 + 1) * 4, :], in_=tp)

            for half in range(2):
                o_ps = mm2p.tile([P, 512], F32, tag="mm2")
                for k in range(16):
                    nc.tensor.matmul(
                        o_ps, lhsT=hqT[:, k, :],
                        rhs=wd_sb[k][:, half * 512:(half + 1) * 512],
                        start=(k == 0), stop=(k == 15),
                    )
                out_sb = outp.tile([P, 512], F32, tag="out")
                nc.scalar.activation(out=out_sb, in_=o_ps, func=ACT.Copy,
                                     scale=stot[:, 0:1])
                nc.sync.dma_start(out=o_d[t][:, half * 512:(half + 1) * 512],
                                  in_=out_sb)

        # software pipeline, depth 2: A(0), A(1), A(2), B(0), A(3), B(1), ...
        DEPTH = 2
        states = {}
        pres = {0: stage_a_pre(0)}
        load_wg(0)
        pres[1] = stage_a_pre(1)
        states[0] = stage_a(0, pres.pop(0))
        load_wd(0)
        for t in range(1, NT):
            if t + 1 < NT:
                pres[t + 1] = stage_a_pre(t + 1)
            states[t] = stage_a(t, pres.pop(t))
            if t == 1:
                load_wg(1)
            if t == NS_TILES - 2:
                load_wd(1)
            if t - DEPTH in states:
                stage_b(t - DEPTH, *states.pop(t - DEPTH))
        for t in sorted(states):
            stage_b(t, *states.pop(t))

    nc.compile()
    return nc


# --------------------------------------------------------------------------
# host orchestration
# --------------------------------------------------------------------------

def _prepare_in_maps(x_flat, w_keep, shared_gate_w, shared_down_w,
                     expert_gate_w, expert_down_w):
    bf = ml_dtypes.bfloat16

    sg_t, sg_mult = _ternarize(shared_gate_w)     # [2I, H]
    sd_t, sd_mult = _ternarize(shared_down_w)     # [H, I]
    wg_s = np.ascontiguousarray(sg_t.T).reshape(8, P, 2 * I).astype(bf)
    wd_s = np.ascontiguousarray(sd_t.T).reshape(16, P, H).astype(bf)

    in_maps = []
    expert_idx = []
    for c in range(E):
        eg_t, eg_mult = _ternarize(expert_gate_w[c])
        ed_t, ed_mult = _ternarize(expert_down_w[c])
        wg_e = np.ascontiguousarray(eg_t.T).reshape(8, P, 2 * I).astype(bf)
        wd_e = np.ascontiguousarray(ed_t.T).reshape(16, P, H).astype(bf)

        idx = np.nonzero(w_keep[:, c] > 0)[0]
        cnt = idx.shape[0]
        expert_idx.append(idx)

        x_all = np.zeros((NT, P, H), np.float32)
        x_all[:NS_TILES] = x_flat[c * 1024:(c + 1) * 1024].reshape(NS_TILES, P, H)
        xe = np.zeros((CAP, H), np.float32)
        xe[:cnt] = x_flat[idx]
        x_all[NS_TILES:] = xe.reshape(NE_TILES, P, H)

        ga = np.zeros((NT, P, 1), np.float32)
        ga[:NS_TILES] = sg_mult / 127.0
        ga[NS_TILES:] = eg_mult / 127.0

        ep = np.zeros((NT, P, 1), np.float32)
        ep[:NS_TILES] = sd_mult / 127.0
        wk = np.zeros((CAP,), np.float32)
        wk[:cnt] = w_keep[idx, c] * (ed_mult / 127.0)
        ep[NS_TILES:] = wk.reshape(NE_TILES, P, 1)

        in_maps.append({
            "x": x_all,
            "ga": ga,
            "ep": ep,
            "wg": np.stack([wg_s, wg_e]),
            "wd": np.stack([wd_s, wd_e]),
        })
    return in_maps, expert_idx


def kernel(x, gate_norm_w, gate_w, shared_gate_w, shared_down_w,
           expert_gate_w, expert_down_w):
    global LAST_EXEC_NS, LAST_WALL_NS
    x = np.asarray(x, np.float32)
    B, S, _ = x.shape
    x_flat = np.ascontiguousarray(x.reshape(B * S, H))

    w_keep = _router(x_flat, np.asarray(gate_norm_w, np.float32),
                     np.asarray(gate_w, np.float32))

    in_maps, expert_idx = _prepare_in_maps(
        x_flat, w_keep,
        np.asarray(shared_gate_w, np.float32),
        np.asarray(shared_down_w, np.float32),
        np.asarray(expert_gate_w, np.float32),
        np.asarray(expert_down_w, np.float32),
    )

    global LAST_IN_MAPS
    LAST_IN_MAPS = in_maps
    nc = _build_nc()
    t0 = time.monotonic()
    try:
        res = run_bass_kernel_spmd(nc, in_maps, list(range(E)), trace=True)
    except Exception:
        res = run_bass_kernel_spmd(nc, in_maps, list(range(E)))
    LAST_WALL_NS = int((time.monotonic() - t0) * 1e9)
    LAST_EXEC_NS = res.exec_time_ns

    out = np.zeros((B * S, H), np.float32)
    for c in range(E):
        o = np.asarray(res.results[c]["o"], np.float32)
        out[c * 1024:(c + 1) * 1024] += o[:NS_TILES].reshape(1024, H)
        idx = expert_idx[c]
        eo = o[NS_TILES:].reshape(CAP, H)[:idx.shape[0]]
        np.add.at(out, idx, eo)
    return out.reshape(B, S, H)
